# revision 1
# baseline (speedup 1.0000x reference)
"""Trainium2 Bass kernel for MultiHeadFAVORAttention.

Sharding: 8 cores, data-parallel over (batch, seq): core j owns batch b=j//4,
tokens [(j%4)*1024, (j%4+1)*1024). The only cross-token coupling (kv / ksum
reductions over S) is handled with one AllReduce per batch half over the
per-head [65, 256] kv^T-augmented matrices (ksum folded in as an extra v
column of ones).

Per-core pipeline (channel-major = [channels, tokens]; fp32 data, fp32r
matmuls; front end processed in two 512-token halves to fit SBUF):
  grouped conv (3 taps) -> multi-scale RFF (cos via explicit range reduction
  + ACT Sin) -> dense proj (q channel-major; k/v token-major) -> RoPE ->
  Nystrom RBF vs landmarks (|kr|^2 folded in via an augmented contraction
  row) -> FAVOR (|kn|^2 via augmented omega column; q-side norm dropped — it
  cancels in num/denom up to eps=1e-6, ~1e-10 relative) -> per-head kv^T ->
  AllReduce -> num/denom -> output dense.
q/k/v intermediates spill to DRAM between phases to fit SBUF.
"""
import os

if os.environ.get("JAX_PLATFORMS", "").strip().lower() == "cpu":
    # bass2jax runs the NEFF through the axon PJRT plugin; a cpu pin would
    # hide the 8 NeuronCores from jax
    os.environ["JAX_PLATFORMS"] = ""

import numpy as np

T = 1024          # tokens per core
HF = 512          # front-end half
C = 1024
H = 16
DH = 64
M = 256           # FAVOR features
G = 3
MS = 256
NCORES = 8
E2 = 66          # padded aug width (fp32r needs even free dims)
MAGIC = float(1.5 * 2 ** 23)
TWO_PI = float(2.0 * np.pi)

_CACHE = {}


def _build(ncores, phases="fekqo"):
    import concourse.bacc as bacc
    import concourse.tile as tile
    from concourse import mybir
    from contextlib import ExitStack

    f32, f32r = mybir.dt.float32, mybir.dt.float32r
    AF = mybir.ActivationFunctionType
    ALU = mybir.AluOpType

    nc = bacc.Bacc("TRN2", target_bir_lowering=False, debug=False,
                   num_devices=ncores)

    # ---- inputs ----
    xT = nc.dram_tensor("xT", [C, T + 2], f32r, kind="ExternalInput")
    convw = nc.dram_tensor("convw", [3, 3, 256, C], f32r, kind="ExternalInput")
    cb = nc.dram_tensor("cb", [128, 3, 8], f32, kind="ExternalInput")
    rffw = nc.dram_tensor("rffw", [3, G, C, MS], f32r, kind="ExternalInput")
    rffb2 = nc.dram_tensor("rffb2", [128, 3, G, 2], f32, kind="ExternalInput")
    projw = nc.dram_tensor("projw", [3, G * MS, C], f32r, kind="ExternalInput")
    outw = nc.dram_tensor("outw", [C, C], f32r, kind="ExternalInput")
    outb = nc.dram_tensor("outb", [128, 8], f32, kind="ExternalInput")
    omegx = nc.dram_tensor("omegx", [128, M], f32r, kind="ExternalInput")
    omegq = nc.dram_tensor("omegq", [128, M], f32r, kind="ExternalInput")
    lmT = nc.dram_tensor("lmT", [E2, 128], f32r, kind="ExternalInput")
    nlm2 = nc.dram_tensor("nlm2", [128, 1], f32, kind="ExternalInput")
    nscal = nc.dram_tensor("nscal", [128, 1], f32, kind="ExternalInput")
    swp = nc.dram_tensor("swp", [128, 128], f32r, kind="ExternalInput")
    kcos = nc.dram_tensor("kcos", [T, DH], f32, kind="ExternalInput")
    ksin = nc.dram_tensor("ksin", [T, DH], f32, kind="ExternalInput")
    qcos = nc.dram_tensor("qcos", [128, T], f32, kind="ExternalInput")
    qsin = nc.dram_tensor("qsin", [128, T], f32, kind="ExternalInput")
    ident = nc.dram_tensor("ident", [128, 128], f32, kind="ExternalInput")

    outT = nc.dram_tensor("outT", [C, T], f32, kind="ExternalOutput")

    rg = [[0, 1, 2, 3], [4, 5, 6, 7]] if ncores == NCORES else [[0]]
    sgam = [float(np.sqrt(2.0 * g)) for g in (0.5, 1.0, 2.0)]

    with tile.TileContext(nc) as tc, ExitStack() as ctx:
        cpool = ctx.enter_context(tc.tile_pool(name="const", bufs=1))
        dpool = ctx.enter_context(tc.tile_pool(name="dram", bufs=1,
                                               space="DRAM"))

        # DRAM spill buffers
        qspill = dpool.tile([8, 128, T], f32r, tag="qspill")
        kaspill = dpool.tile([H, 128, 8, E2], f32, tag="kaspill")
        vaspill = dpool.tile([H, 128, 8, E2], f32r, tag="vaspill")
        kvinA = dpool.tile([8, E2, M], f32, tag="kvinA")
        kvoutA = dpool.tile([8, E2, M], f32, tag="kvoutA")
        kvinB = dpool.tile([8, E2, M], f32, tag="kvinB")
        kvoutB = dpool.tile([8, E2, M], f32, tag="kvoutB")

        # ================= front end (two 512-token halves) =================
        with ExitStack() as fe:
            fpools = {}
            fpools["x"] = fe.enter_context(tc.tile_pool(name="xp", bufs=1))
            fpools["h"] = fe.enter_context(tc.tile_pool(name="hp", bufs=2))
            fpools["f"] = fe.enter_context(tc.tile_pool(name="fp", bufs=2))
            fpools["wc"] = fe.enter_context(tc.tile_pool(name="wc", bufs=3))
            fpools["wr"] = fe.enter_context(tc.tile_pool(name="wr", bufs=3))
            fpools["wp"] = fe.enter_context(tc.tile_pool(name="wp", bufs=1))
            fpools["tb"] = fe.enter_context(tc.tile_pool(name="tb", bufs=1))
            fpools["scr"] = fe.enter_context(tc.tile_pool(name="scr", bufs=2))
            fpools["out"] = fe.enter_context(tc.tile_pool(name="feo", bufs=2))
            psfe = fe.enter_context(
                tc.tile_pool(name="psfe", bufs=4, space="PSUM"))
            pk2 = fe.enter_context(
                tc.tile_pool(name="pk2", bufs=2, space="PSUM"))

            xt = fpools["x"].tile([128, 8, T + 2], f32r, tag="xt")
            nc.sync.dma_start(
                xt[:], xT[:].rearrange("(a p) n -> p a n", p=128))
            cbt = fpools["tb"].tile([128, 3, 8], f32, tag="cbt")
            nc.sync.dma_start(cbt[:], cb[:])
            rffb2t = fpools["tb"].tile([128, 3, G, 2], f32, tag="rffb2t")
            nc.sync.dma_start(rffb2t[:], rffb2[:])

            # constants (idt/omg/... live for the whole program; emitted
            # after x so the conv's gating input transfers first)
            idt = cpool.tile([128, 128], f32, tag="idt")
            nc.sync.dma_start(idt[:], ident[:])
            omg = cpool.tile([128, M], f32r, tag="omg")
            nc.sync.dma_start(omg[:], omegx[:])
            omgq_t = cpool.tile([128, M], f32r, tag="omgq")
            nc.sync.dma_start(omgq_t[:], omegq[:])
            lmt = cpool.tile([E2, 128], f32r, tag="lmt")
            nc.sync.dma_start(lmt[:], lmT[:])
            nlmt2 = cpool.tile([128, 1], f32, tag="nlmt2")
            nc.sync.dma_start(nlmt2[:], nlm2[:])
            nscalt = cpool.tile([128, 1], f32, tag="nscalt")
            nc.sync.dma_start(nscalt[:], nscal[:])
            swpt = cpool.tile([128, 128], f32r, tag="swpt")
            nc.sync.dma_start(swpt[:], swp[:])
            outbt = cpool.tile([128, 8], f32, tag="outbt")
            nc.sync.dma_start(outbt[:], outb[:])
            for i in range(3):
                wct = [fpools["wc"].tile([128, 2, C], f32r, tag="wconv",
                                         name=f"wct{i}_{t_}")
                       for t_ in range(3)]
                for tap in range(3):
                    nc.sync.dma_start(
                        wct[tap][:],
                        convw[i, tap].rearrange("(a p) n -> p a n", p=128))
                wpt = fpools["wp"].tile([128, 6, C], f32r, tag="wproj")
                nc.sync.dma_start(
                    wpt[:], projw[i].rearrange("(a p) n -> p a n", p=128))
                if i == 0:
                    kcost = fpools["tb"].tile([128, 8, DH], f32, tag="kcost")
                    nc.sync.dma_start(
                        kcost[:],
                        kcos[:].rearrange("(c p) d -> p c d", p=128))
                    ksint = fpools["tb"].tile([128, 8, DH], f32, tag="ksint")
                    nc.sync.dma_start(
                        ksint[:],
                        ksin[:].rearrange("(c p) d -> p c d", p=128))
                    qcost = fpools["tb"].tile([128, T], f32, tag="qcost")
                    nc.sync.dma_start(qcost[:], qcos[:])
                    qsint = fpools["tb"].tile([128, T], f32, tag="qsint")
                    nc.sync.dma_start(qsint[:], qsin[:])
                hTs, featss = [], []
                for hf in range(2):
                    t0 = hf * HF
                    # ---- grouped conv -> hT half [C, 512] cm ----
                    hT = fpools["h"].tile([128, 8, HF], f32r, tag="hT",
                                          name=f"hT{i}_{hf}")
                    hTs.append(hT)
                    for coc in range(8):
                        g = coc // 2
                        pc = psfe.tile([128, HF], f32, tag="pfe")
                        n = 0
                        for tap in range(3):
                            for cic in range(2):
                                nc.tensor.matmul(
                                    pc[:],
                                    wct[tap][:, cic, coc * 128:(coc + 1) * 128],
                                    xt[:, g * 2 + cic, t0 + tap:t0 + tap + HF],
                                    start=(n == 0), stop=(n == 5))
                                n += 1
                        nc.scalar.activation(hT[:, coc, :], pc[:], AF.Identity,
                                             bias=cbt[:, i, coc:coc + 1])
                for hf in range(2):
                    t0 = hf * HF
                    hT = hTs[hf]
                    # ---- RFF ----
                    feats = fpools["f"].tile([128, 6, HF], f32r, tag="feats",
                                             name=f"feats{i}_{hf}")
                    featss.append(feats)
                    for g in range(G):
                        if hf == 0:
                            wrt = fpools["wr"].tile([128, 8, MS], f32r,
                                                    tag="wrff",
                                                    name=f"wrt{i}_{g}")
                            nc.sync.dma_start(
                                wrt[:],
                                rffw[i, g].rearrange("(a p) m -> p a m", p=128))
                            if i == 0 and g == 0:
                                wrt_hold = {}
                            wrt_hold[g] = wrt
                        else:
                            wrt = wrt_hold[g]
                        for mc in range(2):
                            pr = psfe.tile([128, HF], f32, tag="pfe")
                            for cc in range(8):
                                nc.tensor.matmul(
                                    pr[:],
                                    wrt[:, cc, mc * 128:(mc + 1) * 128],
                                    hT[:, cc, :],
                                    start=(cc == 0), stop=(cc == 7))
                            # cos(a) = sin(2pi*frac(a/2pi + 1/4))
                            u = fpools["scr"].tile([128, HF], f32, tag="u")
                            nc.vector.tensor_scalar(
                                u[:], pr[:], sgam[g] / TWO_PI,
                                rffb2t[:, i, g, mc:mc + 1],
                                op0=ALU.mult, op1=ALU.add)
                            t1 = fpools["scr"].tile([128, HF], f32, tag="t1")
                            nc.vector.tensor_scalar_add(t1[:], u[:], MAGIC)
                            # (t-M)-u = -frac; sin(-2pi*(-frac)) = sin(2pi*frac)
                            nc.vector.scalar_tensor_tensor(
                                u[:], t1[:], MAGIC, u[:],
                                op0=ALU.subtract, op1=ALU.subtract)
                            nc.scalar.activation(feats[:, g * 2 + mc, :], u[:],
                                                 AF.Sin, scale=-TWO_PI)
                for hf in range(2):
                    t0 = hf * HF
                    feats = featss[hf]
                    # ---- dense proj ----
                    if i == 0:
                        # q channel-major + RoPE, spill per channel-chunk
                        for cc in range(8):
                            pq = psfe.tile([128, HF], f32, tag="pfe")
                            for fc in range(6):
                                nc.tensor.matmul(
                                    pq[:],
                                    wpt[:, fc, cc * 128:(cc + 1) * 128],
                                    feats[:, fc, :],
                                    start=(fc == 0), stop=(fc == 5))
                            qtmp = fpools["scr"].tile([128, HF], f32r, tag="u")
                            nc.scalar.activation(qtmp[:], pq[:], AF.Copy)
                            # rotate-half = partition permutation on the PE
                            psw = psfe.tile([128, HF], f32, tag="pfe")
                            nc.tensor.matmul(psw[:], swpt[:], qtmp[:],
                                             start=True, stop=True)
                            qsw = fpools["scr"].tile([128, HF], f32, tag="t1")
                            nc.vector.tensor_mul(qsw[:], psw[:],
                                                 qsint[:, t0:t0 + HF])
                            nc.gpsimd.tensor_mul(qtmp[:], qtmp[:],
                                                 qcost[:, t0:t0 + HF])
                            qro = fpools["out"].tile([128, HF], f32r, tag="qro")
                            nc.gpsimd.tensor_add(qro[:], qtmp[:], qsw[:])
                            nc.sync.dma_start(qspill[cc, :, t0:t0 + HF],
                                              qro[:])
                    else:
                        # k/v token-major (+RoPE / +ones), spill per chunk
                        for tl in range(4):
                            tcn = hf * 4 + tl
                            pk = pk2.tile([128, T], f32, tag="pk")
                            for p in range(2):
                                for fc in range(6):
                                    nc.tensor.matmul(
                                        pk[:, p * 512:(p + 1) * 512],
                                        feats[:, fc,
                                              tl * 128:(tl + 1) * 128],
                                        wpt[:, fc, p * 512:(p + 1) * 512],
                                        start=(fc == 0), stop=(fc == 5))
                            dst = fpools["out"].tile(
                                [128, H, E2], f32 if i == 1 else f32r,
                                tag="kv65", name=f"kv65_{i}_{tcn}")
                            pkv = pk[:].rearrange("p (h d) -> p h d", d=DH)
                            if i == 1:
                                s1 = fpools["scr"].tile([128, H, DH], f32,
                                                        tag="u")
                                nc.vector.tensor_mul(
                                    s1[:], pkv,
                                    kcost[:, tcn:tcn + 1, :].broadcast_to(
                                        (128, H, DH)))
                                s2 = fpools["scr"].tile([128, H, DH], f32,
                                                        tag="t1")
                                nc.vector.tensor_mul(
                                    s2[:, :, 0:32], pkv[:, :, 32:64],
                                    ksint[:, tcn:tcn + 1, 0:32].broadcast_to(
                                        (128, H, 32)))
                                nc.vector.tensor_mul(
                                    s2[:, :, 32:64], pkv[:, :, 0:32],
                                    ksint[:, tcn:tcn + 1, 32:64].broadcast_to(
                                        (128, H, 32)))
                                nc.gpsimd.tensor_add(dst[:, :, 0:DH], s1[:],
                                                     s2[:])
                                nc.gpsimd.tensor_mul(s1[:], dst[:, :, 0:DH],
                                                     dst[:, :, 0:DH])
                                nc.vector.tensor_reduce(
                                    dst[:, :, DH:DH + 1], s1[:],
                                    axis=mybir.AxisListType.X, op=ALU.add)
                                nc.vector.memset(
                                    dst[:, :, DH + 1:E2].bitcast(f32), 0.0)
                                nc.sync.dma_start(
                                    kaspill[:, :, tcn, :].rearrange(
                                        "h p e -> p h e"),
                                    dst[:])
                            else:
                                nc.scalar.activation(dst[:, :, 0:DH], pkv,
                                                     AF.Copy)
                                nc.vector.memset(
                                    dst[:, :, DH:DH + 1].bitcast(f32), 1.0)
                                nc.vector.memset(
                                    dst[:, :, DH + 1:E2].bitcast(f32), 0.0)
                                nc.sync.dma_start(
                                    vaspill[:, :, tcn, :].rearrange(
                                        "h p e -> p h e"),
                                    dst[:])

        # ================= k side: nystrom + FAVOR + kv =================
        wop = ctx.enter_context(tc.tile_pool(name="wo", bufs=1))
        if "k" not in phases:
            nc.sync.dma_start(outT[:], qspill[0:8, :, :].rearrange(
                "a p n -> (a p) n").bitcast(f32))
            nc.compile()
            return nc
        with ExitStack() as kc:
            kvp = kc.enter_context(tc.tile_pool(name="kvld", bufs=4))
            attp = kc.enter_context(tc.tile_pool(name="attk", bufs=3))
            kpp = kc.enter_context(tc.tile_pool(name="kps", bufs=4))
            smlk = kc.enter_context(tc.tile_pool(name="smlk", bufs=2))
            pTp = kc.enter_context(tc.tile_pool(name="pT", bufs=2,
                                                space="PSUM"))
            pNp = kc.enter_context(tc.tile_pool(name="pN", bufs=1,
                                                space="PSUM"))
            pFp = kc.enter_context(tc.tile_pool(name="pF", bufs=2,
                                                space="PSUM"))
            pKVp = kc.enter_context(tc.tile_pool(name="pKV", bufs=2,
                                                 space="PSUM"))
            for h in range(H):
                kvin = kvinA if h < 8 else kvinB
                kah = kvp.tile([128, 8, E2], f32, tag="kah")
                nc.sync.dma_start(kah[:], kaspill[h])
                vah = kvp.tile([128, 8, E2], f32r, tag="vah")
                nc.sync.dma_start(vah[:], vaspill[h])
                krt = attp.tile([E2, T], f32r, tag="krT")
                for c in range(8):
                    ptp = pTp.tile([E2, 128], f32, tag="pT")
                    nc.tensor.transpose(ptp[:], kah[:, c, :], idt[:])
                    nc.vector.tensor_copy(krt[:, c * 128:(c + 1) * 128],
                                          ptp[:])
                pn = pNp.tile([128, T], f32, tag="pN")
                for p in range(2):
                    nc.tensor.matmul(
                        pn[:, p * 512:(p + 1) * 512],
                        lmt[:], krt[:, p * 512:(p + 1) * 512],
                        start=True, stop=True)
                # rows 0-63 get kn = exp(P/32 - nl/64); rows 64-127 (same P
                # via duplicated landmark columns) get kn^2 = exp(2*(...))
                knx = attp.tile([128, T], f32r, tag="knx")
                nc.scalar.activation(knx[:], pn[:], AF.Exp,
                                     bias=nlmt2[:], scale=nscalt[:])
                pkv_ps = pKVp.tile([E2, M], f32, tag="pKV")
                kps = {}
                for step in range(9):
                    if step < 8:
                        c = step
                        pf = pFp.tile([128, M], f32, tag="pF")
                        nc.tensor.matmul(
                            pf[:], knx[:, c * 128:(c + 1) * 128],
                            omg[:], start=True, stop=True)
                        kpt = kpp.tile([128, M], f32r, tag="kp",
                                       name=f"kp{h}_{c}")
                        nc.scalar.activation(kpt[:], pf[:], AF.Exp)
                        kps[c] = kpt
                    if step >= 1:
                        c = step - 1
                        nc.tensor.matmul(pkv_ps[:], vah[:, c, :],
                                         kps.pop(c)[:],
                                         start=(c == 0), stop=(c == 7))
                kvsb = smlk.tile([E2, M], f32, tag="kvsb")
                nc.vector.tensor_copy(kvsb[:], pkv_ps[:])
                nc.sync.dma_start(kvin[h % 8], kvsb[:])
                if h == 15:
                    nc.gpsimd.collective_compute(
                        "AllReduce", mybir.AluOpType.add, replica_groups=rg,
                        ins=[kvinB.opt()], outs=[kvoutB.opt()])
                if h == 7:
                    nc.gpsimd.collective_compute(
                        "AllReduce", mybir.AluOpType.add, replica_groups=rg,
                        ins=[kvinA.opt()], outs=[kvoutA.opt()])

        # prefetch output dense weights (after all collective-input writes
        # are queued, ~100us before first use in phase O)
        wot = wop.tile([128, 8, C], f32r, tag="wout")
        nc.sync.dma_start(
            wot[:], outw[:].rearrange("(a p) n -> p a n", p=128))

        # ================= q side: FAVOR + num/denom =================
        if "q" not in phases:
            nc.sync.dma_start(outT[:], qspill[0:8, :, :].rearrange(
                "a p n -> (a p) n").bitcast(f32))
            nc.compile()
            return nc
        opool = ctx.enter_context(tc.tile_pool(name="oc", bufs=1))
        ocm = opool.tile([128, 8, T], f32r, tag="ocm")
        with ExitStack() as qc:
            qqp = qc.enter_context(tc.tile_pool(name="qq", bufs=3))
            attq = qc.enter_context(tc.tile_pool(name="attq", bufs=3))
            smlq = qc.enter_context(tc.tile_pool(name="smlq", bufs=2))
            pQ2p = qc.enter_context(tc.tile_pool(name="pQ2", bufs=2,
                                                 space="PSUM"))
            pNump = qc.enter_context(tc.tile_pool(name="pNum", bufs=2,
                                                  space="PSUM"))
            pTqp = qc.enter_context(tc.tile_pool(name="pTq", bufs=2,
                                                 space="PSUM"))
            qq = None
            fronts = {}
            for step in range(H + 1):
              if step < H:
                h = step
                kvout = kvoutA if h < 8 else kvoutB
                if h % 2 == 0:
                    qq = qqp.tile([128, T], f32r, tag="qq",
                                  name=f"qq{h}")
                    nc.sync.dma_start(qq[:], qspill[h // 2])
                kvs = attq.tile([E2, M], f32, tag="kvs", name=f"kvs{h}")
                nc.sync.dma_start(kvs[:], kvout[h % 8])
                kvf = attq.tile([128, 2, E2], f32r, tag="kvf",
                                name=f"kvf{h}")
                for mc in range(2):
                    ptq = pTqp.tile([128, E2], f32, tag="pTq")
                    nc.tensor.transpose(ptq[:], kvs[:, mc * 128:(mc + 1) * 128],
                                        idt[0:E2, 0:E2])
                    nc.vector.tensor_copy(kvf[:, mc, :], ptq[:])
                qpt = attq.tile([128, 2, T], f32r, tag="qp", name=f"qpt{h}")
                hb = (h % 2) * 64
                for mc in range(2):
                    for p in range(2):
                        pq2 = pQ2p.tile([128, HF], f32, tag="pQ2")
                        nc.tensor.matmul(
                            pq2[:],
                            omgq_t[hb:hb + 64, mc * 128:(mc + 1) * 128],
                            qq[hb:hb + 64, p * 512:(p + 1) * 512],
                            start=True, stop=True)
                        nc.scalar.activation(
                            qpt[:, mc, p * 512:(p + 1) * 512], pq2[:], AF.Exp)
                fronts[h] = (kvf, qpt)
              if step >= 1:
                h = step - 1
                hb = (h % 2) * 64
                kvf, qpt = fronts.pop(h)
                pnum = pNump.tile([E2, T], f32, tag="pNum")
                for mc in range(2):
                    for p in range(2):
                        nc.tensor.matmul(
                            pnum[:, p * 512:(p + 1) * 512],
                            kvf[:, mc, :],
                            qpt[:, mc, p * 512:(p + 1) * 512],
                            start=(mc == 0), stop=(mc == 1))
                drow = smlq.tile([E2, T], f32, tag="drow")
                nc.vector.reciprocal(drow[64:65, :], pnum[64:65, :])
                rc = smlq.tile([1, T], f32, tag="rcp")
                nc.sync.dma_start(rc[0:1, :], drow[64:65, :])
                rb = smlq.tile([64, T], f32, tag="rb")
                nc.gpsimd.partition_broadcast(rb[:], rc[0:1, :])
                if hb == 0:
                    nc.vector.tensor_mul(ocm[0:64, h // 2, :],
                                         pnum[0:64, :], rb[:])
                else:
                    osc = smlq.tile([64, T], f32r, tag="osc")
                    nc.vector.tensor_mul(osc[:], pnum[0:64, :], rb[:])
                    nc.sync.dma_start(ocm[64:128, h // 2, :], osc[:])

        # ================= output dense =================
        if "o" not in phases:
            nc.sync.dma_start(outT[:], qspill[0:8, :, :].rearrange(
                "a p n -> (a p) n").bitcast(f32))
            nc.compile()
            return nc
        with (
            tc.tile_pool(name="psO", bufs=3, space="PSUM") as psO,
            tc.tile_pool(name="oto", bufs=2) as otop,
        ):
            for coc in range(8):
                po = psO.tile([128, T], f32, tag="pO")
                for p in range(2):
                    for cc in range(8):
                        nc.tensor.matmul(
                            po[:, p * 512:(p + 1) * 512],
                            wot[:, cc, coc * 128:(coc + 1) * 128],
                            ocm[:, cc, p * 512:(p + 1) * 512],
                            start=(cc == 0), stop=(cc == 7))
                ot = otop.tile([128, T], f32, tag="ot")
                nc.scalar.activation(ot[:], po[:], AF.Identity,
                                     bias=outbt[:, coc:coc + 1])
                nc.sync.dma_start(outT[coc * 128:(coc + 1) * 128, :], ot[:])

    nc.compile()
    return nc


def _host_prep(x, conv_k, conv_b, rff_w, rff_b, proj_w, proj_b, omega,
               landmarks, out_w, out_b):
    """Shared + per-core input arrays (all float32)."""
    f32 = np.float32
    assert not np.any(proj_b), "kernel assumes proj_b == 0 (spec: zeros)"
    S = x.shape[1]

    # rope tables in fp32 arithmetic to match the jax fp32 reference
    inv = (1.0 / (10000.0 ** (np.arange(0, DH, 2, dtype=f32) / f32(DH)))).astype(f32)
    fmat = np.arange(S, dtype=f32)[:, None] * inv[None, :]
    emb = np.concatenate([fmat, fmat], axis=1).astype(f32)     # [S, 64]
    sin_t, cos_t = np.sin(emb), np.cos(emb)
    sgn = np.concatenate([-np.ones(DH // 2, f32), np.ones(DH // 2, f32)])
    sinS = sin_t * sgn[None, :]

    shared = {
        "convw": np.ascontiguousarray(conv_k, f32),
        "cb": np.ascontiguousarray(
            conv_b.reshape(3, 8, 128).transpose(2, 0, 1), f32),
        "rffw": np.ascontiguousarray(rff_w, f32),
        "rffb2": np.ascontiguousarray(
            (rff_b / (2.0 * np.pi) + 0.25).reshape(3, G, 2, 128)
            .transpose(3, 0, 1, 2), f32),
        "projw": np.ascontiguousarray(proj_w * np.sqrt(2.0 / MS), f32),
        "outw": np.ascontiguousarray(out_w, f32),
        "outb": np.ascontiguousarray(out_b.reshape(8, 128).T, f32),
        "ident": np.eye(128, dtype=f32),
    }
    nl = (landmarks.astype(f32) ** 2).sum(1)[:, None] / f32(DH)
    shared["nlm2"] = np.concatenate([-nl, -2.0 * nl], 0).astype(f32)
    shared["nscal"] = np.concatenate(
        [np.full((64, 1), 1.0 / 32.0, f32), np.full((64, 1), 2.0 / 32.0, f32)], 0)
    swp = np.zeros((128, 128), f32)
    for blk in range(2):
        for d in range(32):
            swp[blk * 64 + d + 32, blk * 64 + d] = -1.0
            swp[blk * 64 + d, blk * 64 + d + 32] = 1.0
    shared["swp"] = swp
    omegx = np.full((128, M), -0.5, f32)
    omegx[0:DH, 0:M] = omega
    shared["omegx"] = omegx
    shared["omegq"] = np.ascontiguousarray(
        np.concatenate([omega, omega], axis=0), f32)
    lmTa = np.zeros((E2, DH), f32)
    lmTa[0:DH] = landmarks.T
    lmTa[DH] = -0.5
    shared["lmT"] = np.ascontiguousarray(
        np.concatenate([lmTa, lmTa], axis=1), f32)

    per_core = []
    for j in range(NCORES):
        b, s0 = j // 4, (j % 4) * T
        xp = np.pad(x[b], ((1, 1), (0, 0)))
        m = dict(shared)
        m["xT"] = np.ascontiguousarray(xp[s0:s0 + T + 2].T, f32)
        m["kcos"] = np.ascontiguousarray(cos_t[s0:s0 + T], f32)
        m["ksin"] = np.ascontiguousarray(sinS[s0:s0 + T], f32)
        m["qcos"] = np.ascontiguousarray(
            np.tile(cos_t[s0:s0 + T].T, (2, 1)), f32)
        # unsigned: the rotate-half signs live in the swp permutation matrix
        m["qsin"] = np.ascontiguousarray(
            np.tile(sin_t[s0:s0 + T].T, (2, 1)), f32)
        per_core.append(m)
    return per_core


def kernel(x, conv_k, conv_b, rff_w, rff_b, proj_w, proj_b, omega, landmarks,
           out_w, out_b):
    from concourse.bass_utils import run_bass_kernel_spmd

    if "nc" not in _CACHE:
        _CACHE["nc"] = _build(NCORES)
    nc = _CACHE["nc"]
    in_maps = _host_prep(
        np.asarray(x, np.float32), np.asarray(conv_k, np.float32),
        np.asarray(conv_b, np.float32), np.asarray(rff_w, np.float32),
        np.asarray(rff_b, np.float32), np.asarray(proj_w, np.float32),
        np.asarray(proj_b, np.float32), np.asarray(omega, np.float32),
        np.asarray(landmarks, np.float32), np.asarray(out_w, np.float32),
        np.asarray(out_b, np.float32))
    res = run_bass_kernel_spmd(nc, in_maps, core_ids=list(range(NCORES)))
    out = np.empty((2, 4096, C), np.float32)
    for j in range(NCORES):
        b, s0 = j // 4, (j % 4) * T
        out[b, s0:s0 + T] = res.results[j]["outT"].T
    return out

